# revision 5
# baseline (speedup 1.0000x reference)
"""Causal multi-head attention on 8 TRN2 NeuronCores, data-parallel over batch.

Per-core work (batch=1): q/k/v projections, per-head causal softmax
attention. All matmuls in fp16 (f32 PSUM accumulation); softmax max/exp in
f32. The P^T needed by the attention*V matmul comes from the DMA xbar
transpose engine (dma_start_transpose) instead of PE transposes + DVE
PSUM->SBUF copies, freeing both engines. Softmax denominators come from a
ones-column appended to V, so Scalar never touches the accumulator; the
final normalize runs on GpSimd from an SBUF copy of the AV accumulators.
Host-side prep: inputs transposed to [D_IN, L] and cast to fp16.
"""

import sys

sys.path.insert(0, "/opt/trn_rl_repo")

import numpy as np

import concourse.bacc as bacc
import concourse.tile as tile
from concourse import mybir
from concourse.bass_utils import run_bass_kernel_spmd
from concourse.masks import make_causal_mask, make_identity

B, L, DIN, H, D = 8, 1024, 512, 8, 64
HD = H * D
F32 = mybir.dt.float32
F16 = mybir.dt.float16
N_CORES = 8
MASK_VAL = -60000.0

_cached = {}


def _build():
    nc = bacc.Bacc("TRN2", target_bir_lowering=False, debug=False,
                   enable_asserts=False, num_devices=N_CORES)

    qt_d = nc.dram_tensor("qt", [DIN, L], F16, kind="ExternalInput").ap()
    kt_d = nc.dram_tensor("kt", [DIN, L], F16, kind="ExternalInput").ap()
    vt_d = nc.dram_tensor("vt", [DIN, L], F16, kind="ExternalInput").ap()
    wq_d = nc.dram_tensor("wq", [DIN, HD], F16, kind="ExternalInput").ap()
    wk_d = nc.dram_tensor("wk", [DIN, HD], F16, kind="ExternalInput").ap()
    wv_d = nc.dram_tensor("wv", [DIN, HD], F16, kind="ExternalInput").ap()
    out_d = nc.dram_tensor("out", [L, HD], F32, kind="ExternalOutput").ap()

    with tile.TileContext(nc) as tc:
        _body(tc, out_d, qt_d, kt_d, vt_d, wq_d, wk_d, wv_d)
    nc.compile()
    return nc


def _body(tc, out_d, qt_d, kt_d, vt_d, wq_d, wk_d, wv_d):
    nc = tc.nc
    from contextlib import ExitStack
    with ExitStack() as ctx:
        const = ctx.enter_context(tc.tile_pool(name="const", bufs=1))
        big = ctx.enter_context(tc.tile_pool(name="big", bufs=1))
        sb = ctx.enter_context(tc.tile_pool(name="sb", bufs=6))
        # PSUM: 3x S (f32, 2 banks each) + av_a + av_b (1 bank each) = 8
        ps_s = ctx.enter_context(tc.tile_pool(name="pss", bufs=3, space="PSUM"))
        ps_av = ctx.enter_context(tc.tile_pool(name="psav", bufs=1, space="PSUM"))

        ident = const.tile([128, 128], F16)
        make_identity(nc, ident[:])
        cmaskT = const.tile([128, 128], F16)
        nc.gpsimd.memset(cmaskT[:], MASK_VAL)
        nc.gpsimd.affine_select(
            out=cmaskT[:], in_=cmaskT[:],
            compare_op=mybir.AluOpType.is_gt, fill=0.0,
            base=0, pattern=[[-1, 128]], channel_multiplier=1)

        # ---- load inputs (transposed, fp16), one DMA per DIN-chunk piece
        xq = big.tile([128, 4, L], F16)
        xk = big.tile([128, 4, L], F16)
        xv = big.tile([128, 4, L], F16)
        wq = big.tile([128, 4, HD], F16)
        wk = big.tile([128, 4, HD], F16)
        wv = big.tile([128, 4, HD], F16)
        for t, d in ((wq, wq_d), (xq, qt_d), (wk, wk_d), (xk, kt_d),
                     (wv, wv_d), (xv, vt_d)):
            r = d.rearrange("(c p) l -> p c l", p=128)
            for c in range(4):
                nc.sync.dma_start(t[:, c, :], r[:, c, :])

        # v65: V rows with a ones column appended per head; the attention*V
        # matmul's 65th output column is then the softmax denominator.
        v65 = big.tile([128, 8, 8, 65], F16)
        nc.gpsimd.memset(v65[:, :, :, 64:65], 1.0)

        # PE warm-up: dummy matmuls while the loads stream in, so the
        # clock governor sees sustained activity before projections.
        warm = const.tile([128, 512], F16)
        nc.vector.memset(warm[:], 0.0)
        wps = ps_s.tile([128, 1024], F32, tag="S")
        for i in range(16):
            nc.tensor.matmul(wps[:, 0:512], lhsT=warm[:, 0:128], rhs=warm[:],
                             start=(i == 0), stop=(i == 15))

        # ---- projections (fp16 matmuls, f32 psum)
        # qT/kT "transposed" form [hd, L]; v natural [L, hd]
        qTs = big.tile([128, 4, L], F16)
        kTs = big.tile([128, 4, L], F16)
        for w_sb, x_sb, dst in ((wq, xq, qTs), (wk, xk, kTs)):
            for t in range(4):
                for s in range(2):
                    pp = ps_s.tile([128, 512], F32, tag="S")
                    for c in range(4):
                        nc.tensor.matmul(
                            pp[:],
                            lhsT=w_sb[:, c, t * 128:(t + 1) * 128],
                            rhs=x_sb[:, c, s * 512:(s + 1) * 512],
                            start=(c == 0), stop=(c == 3))
                    if dst is qTs:
                        nc.scalar.copy(dst[:, t, s * 512:(s + 1) * 512], pp[:])
                    else:
                        nc.vector.tensor_copy(dst[:, t, s * 512:(s + 1) * 512], pp[:])

        def emit_vproj():
            for lt in range(8):
                pp = ps_s.tile([128, 512], F32, name=f"ppv{lt}", tag="S")
                for c in range(4):
                    nc.tensor.matmul(
                        pp[:],
                        lhsT=xv[:, c, lt * 128:(lt + 1) * 128],
                        rhs=wv[:, c, :],
                        start=(c == 0), stop=(c == 3))
                dstv = v65[:, lt, :, 0:64]
                srcv = pp[:].rearrange("p (h d) -> p h d", h=8)
                if lt % 2 == 0:
                    nc.vector.tensor_copy(dstv, srcv)
                else:
                    nc.scalar.copy(dstv, srcv)

        # ---- attention (heavy q-tiles first: they catch the warm clock)
        for qt in range(7, -1, -1):
            Lq0 = qt * 128
            Lk = (qt + 1) * 128
            nkc = qt + 1

            # per head: scores -> max -> exp -> xbar transpose. The PE
            # queue gets long runs of S matmuls; max/exp/transpose of head
            # h run on DVE/Scalar/DMA while the PE streams heads h+1...
            pTss = []
            for h in range(8):
                t, po = h // 2, (h % 2) * 64
                S = ps_s.tile([128, 1024], F32, name=f"S{qt}_{h}", tag="S")
                for w in range(0, Lk, 512):
                    n = min(512, Lk - w)
                    diag = (w + n == Lk)
                    nc.tensor.matmul(
                        S[:, w:w + n],
                        lhsT=qTs[po:po + 64, t, Lq0:Lq0 + 128],
                        rhs=kTs[po:po + 64, t, w:w + n],
                        start=True, stop=not diag)
                    if diag:
                        nc.tensor.matmul(S[:, Lk - 128:Lk], lhsT=cmaskT[:],
                                         rhs=ident[:], start=False, stop=True)
                if qt == 7 and h == 0:
                    emit_vproj()  # fills the proj->attention PE gap
                nm = sb.tile([128, 1], F32, name=f"nm{qt}_{h}", tag="nm",
                             bufs=12)
                nc.vector.reduce_max(nm[:], S[:, :Lk], axis=mybir.AxisListType.X,
                                     negate=True)
                pr = sb.tile([128, 1024], F16, name=f"pr{qt}_{h}", tag="pr",
                             bufs=8)
                nc.scalar.activation(pr[:, :Lk], S[:, :Lk],
                                     mybir.ActivationFunctionType.Exp,
                                     bias=nm[:], scale=1.0)
                pTs = sb.tile([128, 8, 128], F16, name=f"pTs{qt}_{h}",
                              tag="pTs", bufs=8)
                nc.sync.dma_start_transpose(pTs[:, :nkc, :], pr[:, :Lk])
                pTss.append(pTs)

            av_a = ps_av.tile([128, 4, 65], F32, name=f"ava{qt}", tag="av_a")
            av_b = ps_av.tile([128, 4, 65], F32, name=f"avb{qt}", tag="av_b")
            avs = (av_a, av_b)
            for h in range(8):
                av = avs[h // 4]
                for kc in range(nkc):
                    nc.tensor.matmul(
                        av[:, h % 4, :],
                        lhsT=pTss[h][:, kc, :],
                        rhs=v65[:, kc, h, :],
                        start=(kc == 0), stop=(kc == qt))

            # evacuate AV to SBUF (Scalar), normalize on GpSimd (idle), DMA out
            avsb = sb.tile([128, 8, 65], F32, name=f"avsb{qt}", tag="avsb",
                           bufs=2)
            nc.scalar.copy(avsb[:, 0:4, :], av_a[:])
            nc.scalar.copy(avsb[:, 4:8, :], av_b[:])
            rec = sb.tile([128, 8], F32, tag="rec")
            nc.vector.reciprocal(rec[:], avsb[:, :, 64])
            out_sb = sb.tile([128, HD], F32, tag="osb")
            nc.gpsimd.tensor_mul(
                out_sb[:].rearrange("p (h d) -> p h d", h=8),
                avsb[:, :, 0:64],
                rec[:].unsqueeze(2).broadcast_to([128, 8, 64]))
            nc.sync.dma_start(out_d[Lq0:Lq0 + 128, :], out_sb[:])


def kernel(Q_seq, K_seq, V_seq, WQ, WK, WV, _trace=False):
    if "nc" not in _cached:
        _cached["nc"] = _build()
    nc = _cached["nc"]

    wq16 = (np.asarray(WQ, dtype=np.float32) * 0.125).astype(np.float16)
    wk16 = np.asarray(WK, dtype=np.float16)
    wv16 = np.asarray(WV, dtype=np.float16)
    in_maps = []
    for b in range(N_CORES):
        in_maps.append({
            "qt": np.ascontiguousarray(np.asarray(Q_seq[b]).T.astype(np.float16)),
            "kt": np.ascontiguousarray(np.asarray(K_seq[b]).T.astype(np.float16)),
            "vt": np.ascontiguousarray(np.asarray(V_seq[b]).T.astype(np.float16)),
            "wq": wq16, "wk": wk16, "wv": wv16,
        })
    res = run_bass_kernel_spmd(nc, in_maps, core_ids=list(range(N_CORES)),
                               trace=_trace)
    out = np.stack([res.results[b]["out"] for b in range(N_CORES)], axis=0)
    if _trace:
        kernel.last_exec_time_ns = res.exec_time_ns
        kernel.last_results = res
    return out
